# revision 20
# baseline (speedup 1.0000x reference)
"""Trainium2 kernel for nn_ClusterManager (vq_codebook).

Strategy
--------
The only heavy compute in the module is the per-batch feature Gram matrix
G_b = ff_b @ ff_b.T with ff_b = features[b].reshape(256, 16384) (fp32):
~17 GFLOP total.  Everything else (FPS over 256x256 distances, capacity
assignment over 256 channels) is a few hundred KFLOPs of inherently
sequential argmax/scan logic, done on host in fp64.

Data-parallel over batch: core b computes batch b's Gram matrix.

Precision: FPS argmax decision margins on this problem are ~0.18 in
squared-distance units (d2 scale ~3e4), so single-pass fp16/bf16
(err ~0.1) would flip decisions.  Scheme:
    x  = hi + lo            hi = fp16(x), lo8 = fp8e4m3(lo * 4096)
    G  = hi@hi.T  +  (S + S.T)/4096        S = a@lo8.T,  a = fp8(hi)
The a-for-hi substitution in S only costs |(hi-a)@lo| ~ 5e-3 and lets S
run as an fp8 DoubleRow matmul: contraction 256 per instruction (2
k-tiles), measured 2x MAC throughput vs fp16.  hh stays exact fp16.
Max |d2| error ~0.013 -- 14x below the decision margin.

Per-core pipeline over 128 k-tiles (k = SBUF partition dim, 128/tile):
  DMA hi (fp16, sync queue) + lo8 (fp8, gpsimd queue) per group
  DVE casts a = fp8(hi) (no other decode exists; lo is used raw)
  PE per k-tile:   mm(hh[:, :256],  lhsT=hi0, rhs=hi,  N=256)  fp16
                   mm(hh[:, 256:],  lhsT=hi1, rhs=hi1, N=128)  fp16
     per k-pair:   mm(S[:, :256],   lhsT=a0,  rhs=lo,  N=256)  fp8 DR
                   mm(S[:, 256:],   lhsT=a1,  rhs=lo,  N=256)  fp8 DR
  accumulating in PSUM over all k.  hh's lower-left block is restored
  by symmetry on host; S is used as S + S.T on host.
A few dummy matmuls on zeroed SBUF run during the DMA fill so the PE's
HAM clock-gate is already at 8/8 when real data lands.
"""

import os

import numpy as np

DEBUG_NO_S = bool(os.environ.get("DEBUG_NO_S"))
DEBUG_NO_WARMUP = bool(os.environ.get("DEBUG_NO_WARMUP"))

# ---------------------------------------------------------------- constants
B = 8
C = 256
DF = 16384  # 64 * 256 flattened feature dim
P = 128
KT = DF // P          # 128 k-tiles
LO_SCALE = 4096.0     # lo is stored as fp8e4m3 of lo*2^12; host divides S by it
WARMUP_MM = 6         # dummy N=512 fp16 matmuls issued before real work
# k-tile counts per pipeline group (must be even: S pairs can't straddle
# groups).  Small groups first so the first matmul's DMA chain is short.
GROUP_SIZES = [2, 2, 4] + [8] * 15
assert sum(GROUP_SIZES) == KT
LAG = 3               # groups the matmuls trail the DMA stage
BUFS = 8              # tile-pool depth (SBUF is plentiful)

NUM_CLUSTERS = 16
UPDATE_RATE = 0.2
_BASE = C // NUM_CLUSTERS
_REM = C % NUM_CLUSTERS
CLUSTER_SIZES = np.array(
    [_BASE + 1] * _REM + [_BASE] * (NUM_CLUSTERS - _REM), dtype=np.int64
)

_CACHED = {}


# ---------------------------------------------------------------- device part
def _build_program():
    import concourse.tile as tile
    from concourse import bacc, mybir

    f32 = mybir.dt.float32
    f16 = mybir.dt.float16
    f8 = mybir.dt.float8e4
    DR = mybir.MatmulPerfMode.DoubleRow

    nc = bacc.Bacc(
        "TRN2",
        target_bir_lowering=False,
        debug=False,
        enable_asserts=False,
        num_devices=B,
    )

    # input layout: element [p, kt, c] = term[c, kt*128 + p]
    xhi = nc.dram_tensor("xhi", [P, KT, C], f16, kind="ExternalInput").ap()
    xlo = nc.dram_tensor("xlo", [P, KT, C], f8, kind="ExternalInput").ap()
    # hh needs fp32 (values ~1.7e4, ulp matters); S is ~700-scale stored and
    # fp16 is plenty.  Separate dram tensors so the tail DMAs run on separate
    # queues (sync + scalar).
    ghh = nc.dram_tensor("ghh", [P, 3 * P], f32, kind="ExternalOutput").ap()
    gs16 = nc.dram_tensor("gs16", [P, 4 * P], f16, kind="ExternalOutput").ap()

    # PE warm-up, emitted BEFORE the TileContext scope: every in-scope
    # instruction waits for the all-engine scope-begin barrier (~7.1us into
    # the NEFF), but pre-scope instructions run as soon as each engine's
    # instruction stream is loaded (~5.5us).  The HAM clock gate needs
    # ~3.4us of sustained PE busy to lift the PE from 1.2 to 2.4 GHz, so
    # these dummies (on uninitialized SBUF, into a scratch PSUM bank that is
    # never read) make the real matmul stream start warm.  The PE engine
    # queue is 64 deep, so the PE sequencer still reaches the scope barrier
    # immediately; the array crunches dummies while DMA setup proceeds.
    warm_ctx = nc.sbuf_tensor([P, 4 * P], f16)
    wrm = warm_ctx.__enter__()
    warm_psum_ctx = nc.psum_tensor([P, 4 * P], f32)
    ps_w = warm_psum_ctx.__enter__()
    if not DEBUG_NO_WARMUP:
        for _ in range(WARMUP_MM):
            nc.tensor.matmul(
                ps_w.ap(), lhsT=wrm.ap()[:, :P], rhs=wrm.ap(), start=True,
                stop=True, skip_group_check=True,
            )

    with tile.TileContext(nc) as tc:
        with (
            tc.tile_pool(name="hi", bufs=BUFS) as hi_pool,
            tc.tile_pool(name="lo", bufs=BUFS) as lo_pool,
            tc.tile_pool(name="a8", bufs=BUFS) as a_pool,
            tc.tile_pool(name="gacc", bufs=1, space="PSUM") as gacc_pool,
            tc.tile_pool(name="gout", bufs=1) as gout_pool,
        ):
            # one PSUM accumulation chain per 2KB bank: start_tensor_calc
            # lazily zeroes the whole 2KB-aligned region, so chains must not
            # share a bank ([P, 512] f32 = exactly one bank each).
            ps_hh0 = gacc_pool.tile([P, 4 * P], f32, tag="hh0", name="ps_hh0")
            ps_hh1 = gacc_pool.tile([P, 4 * P], f32, tag="hh1", name="ps_hh1")
            ps_s0 = gacc_pool.tile([P, 4 * P], f32, tag="s0", name="ps_s0")
            ps_s1 = gacc_pool.tile([P, 4 * P], f32, tag="s1", name="ps_s1")

            def stage_hi(k0, kn):
                hi = hi_pool.tile([P, kn, C], f16, tag="hi")
                nc.sync.dma_start(hi[:], xhi[:, k0 : k0 + kn, :])
                return hi

            def stage_lo(k0, kn, hi):
                lo = lo_pool.tile([P, kn, C], f8, tag="lo")
                nc.gpsimd.dma_start(lo[:], xlo[:, k0 : k0 + kn, :])
                a8t = a_pool.tile([P, kn, C], f8, tag="a8")
                # a = fp8(hi), RNE; one DVE cast per k-pair so the first S
                # matmul of the group isn't gated on the whole group's cast
                for j in range(kn // 2):
                    nc.vector.tensor_copy(
                        a8t[:, 2 * j : 2 * j + 2, :], hi[:, 2 * j : 2 * j + 2, :]
                    )
                return lo, a8t

            def mm_hh(kt_local, k_idx, hi):
                nc.tensor.matmul(
                    ps_hh0[:, : 2 * P],
                    lhsT=hi[:, kt_local, 0:P],
                    rhs=hi[:, kt_local, :],
                    start=k_idx == 0,
                    stop=k_idx == KT - 1,
                    skip_group_check=True,
                )
                nc.tensor.matmul(
                    ps_hh1[:, :P],
                    lhsT=hi[:, kt_local, P : 2 * P],
                    rhs=hi[:, kt_local, P : 2 * P],
                    start=k_idx == 0,
                    stop=k_idx == KT - 1,
                    skip_group_check=True,
                )

            def mm_s(j, k_pair, lo, a8t):
                for m in range(2):
                    nc.tensor.matmul(
                        (ps_s0 if m == 0 else ps_s1)[:, : 2 * P],
                        lhsT=a8t[:, 2 * j : 2 * j + 2, m * P : (m + 1) * P],
                        rhs=lo[:, 2 * j : 2 * j + 2, :],
                        start=k_pair == 0,
                        stop=k_pair == KT - 2,
                        perf_mode=DR,
                        skip_group_check=True,
                    )

            def matmuls(k0, kn, hi, lo, a8t, last=False):
                if last:
                    # S first so its output casts overlap the remaining hh MMs
                    for j in range(kn // 2):
                        mm_s(j, k0 + 2 * j, lo, a8t)
                    for kt in range(kn):
                        mm_hh(kt, k0 + kt, hi)
                else:
                    # hh first: it only needs the hi DMA, giving the DVE a
                    # full hh-block of slack to finish the group's a-casts
                    for kt in range(kn):
                        mm_hh(kt, k0 + kt, hi)
                    for j in range(kn // 2):
                        mm_s(j, k0 + 2 * j, lo, a8t)

            starts = [0]
            for kn in GROUP_SIZES[:-1]:
                starts.append(starts[-1] + kn)
            ngrp = len(GROUP_SIZES)
            # hi-DMAs issue one group ahead of lo-DMAs: the hh matmuls (the
            # head of each group) only need hi, so hi descriptors get ring
            # priority; lo for group g lands while group g's hh block runs.
            his = {}
            los = {}
            done = 0
            for gi in range(ngrp + LAG):
                if gi < ngrp:
                    his[gi] = stage_hi(starts[gi], GROUP_SIZES[gi])
                lg = gi - 1
                if 0 <= lg < ngrp:
                    los[lg] = stage_lo(starts[lg], GROUP_SIZES[lg], his[lg])
                if gi >= LAG:
                    g = gi - LAG
                    matmuls(starts[g], GROUP_SIZES[g], his.pop(g), *los.pop(g),
                            last=g == ngrp - 1)

            # ghh cols: [hh(0,:)(256) | hh(1,1)(128)]; gs16: S (a@lo, x4096).
            # S casts first (their MMs finished before the last hh MMs) and
            # split across DVE+ACT; the gs16 DMA issues while hh still runs.
            g_sb16 = gout_pool.tile([P, 4 * P], f16, tag="gsb16")
            nc.vector.tensor_copy(g_sb16[:, : 2 * P], ps_s0[:, : 2 * P])
            nc.scalar.copy(g_sb16[:, 2 * P :], ps_s1[:, : 2 * P])
            nc.scalar.dma_start(gs16[:], g_sb16[:])
            g_sb32 = gout_pool.tile([P, 3 * P], f32, tag="gsb32")
            nc.scalar.copy(g_sb32[:, : 2 * P], ps_hh0[:, : 2 * P])
            nc.vector.tensor_copy(g_sb32[:, 2 * P :], ps_hh1[:, :P])
            nc.sync.dma_start(ghh[:], g_sb32[:])

    warm_psum_ctx.__exit__(None, None, None)
    warm_ctx.__exit__(None, None, None)
    nc.compile()
    return nc


def _device_layout(ff_b):
    """[C, DF] fp32 -> (hi [P,KT,C] fp16, lo8 [P,KT,C] fp8e4m3 of lo*4096)."""
    import ml_dtypes

    hi = ff_b.astype(np.float16)
    lo8 = ((ff_b - hi.astype(np.float32)) * LO_SCALE).astype(ml_dtypes.float8_e4m3)
    hi_t = np.ascontiguousarray(hi.reshape(C, KT, P).transpose(2, 1, 0))
    lo_t = np.ascontiguousarray(lo8.reshape(C, KT, P).transpose(2, 1, 0))
    return hi_t, lo_t


def _run_device(ff, trace=False, trace_cores=None):
    """ff: [B, C, DF] fp32 -> (Ghh [B,C,C], S [B,C,C], BassKernelResults).

    Ghh's lower-left 128x128 block is not computed on device; it is
    restored from the upper-right block by symmetry here.  S = a@lo.T
    (a = fp8(hi)); G = Ghh + S + S.T after the host divides by LO_SCALE.
    """
    from concourse.bass_utils import run_bass_kernel_spmd

    if "nc" not in _CACHED:
        _CACHED["nc"] = _build_program()
    nc = _CACHED["nc"]

    in_maps = []
    for b in range(B):
        hi_t, lo_t = _device_layout(ff[b])
        in_maps.append({"xhi": hi_t, "xlo": lo_t})
    res = run_bass_kernel_spmd(
        nc, in_maps, core_ids=list(range(B)), trace=trace, trace_cores=trace_cores
    )
    g32 = np.stack([res.results[b]["ghh"] for b in range(B)])  # [B, P, 3P] f32
    g16 = np.stack([res.results[b]["gs16"] for b in range(B)])  # [B, P, 4P] f16
    Ghh = np.empty((B, C, C), np.float32)
    Ghh[:, :P, :] = g32[:, :, : 2 * P]
    Ghh[:, P:, P:] = g32[:, :, 2 * P :]
    Ghh[:, P:, :P] = np.swapaxes(Ghh[:, :P, P:], 1, 2)
    S = np.empty((B, C, C), np.float32)
    S[:, :P, :] = g16[:, :, : 2 * P]
    S[:, P:, :] = g16[:, :, 2 * P :]
    S /= LO_SCALE
    return Ghh, S, res


# ---------------------------------------------------------------- host part
def _cdist(a, b):
    d2 = (
        np.sum(a * a, -1)[..., :, None]
        + np.sum(b * b, -1)[..., None, :]
        - 2.0 * (a @ np.swapaxes(b, -1, -2))
    )
    return np.sqrt(np.clip(d2, 0.0, None))


def _fps_from_D(D, k):
    start = int(np.argmax(D.sum(1)))
    sel = [start]
    min_d = D[start].copy()
    for _ in range(k - 1):
        far = int(np.argmax(min_d))
        sel.append(far)
        min_d = np.minimum(min_d, D[far])
    return np.array(sel)


def _capacity_assign(D, sizes):
    order = np.argsort(D, axis=1, kind="stable")  # [C, K]
    counts = np.zeros(sizes.shape[0], np.int64)
    out = np.empty(D.shape[0], np.int32)
    for ci in range(D.shape[0]):
        row = order[ci]
        chosen = row[int(np.argmax(counts[row] < sizes[row]))]
        counts[chosen] += 1
        out[ci] = chosen
    return out


def _finish(d2_batches, pos_emb_batch):
    pos_emb = pos_emb_batch.astype(np.float64)
    K = NUM_CLUSTERS
    pos = pos_emb[0]
    centers = pos[_fps_from_D(_cdist(pos, pos), K)]
    sels = []
    for bi in range(B):
        d2 = d2_batches[bi].copy()
        np.fill_diagonal(d2, 0.0)
        sels.append(_fps_from_D(np.sqrt(np.clip(d2, 0.0, None)), K))
    sel = np.stack(sels)
    center_coords = pos_emb[np.arange(B)[:, None], sel]
    temp_assign = np.argmin(_cdist(pos_emb, center_coords), -1)
    flat_a = temp_assign.reshape(-1)
    flat_p = pos_emb.reshape(-1, 3)
    sums = np.zeros((K, 3))
    cnts = np.zeros(K)
    np.add.at(sums, flat_a, flat_p)
    np.add.at(cnts, flat_a, 1.0)
    avg = np.where(cnts[:, None] > 0, sums / np.maximum(cnts, 1.0)[:, None], 0.0)
    matching = np.argmin(_cdist(centers, avg), axis=1)
    centers = (1.0 - UPDATE_RATE) * centers + UPDATE_RATE * avg[matching]
    return _capacity_assign(_cdist(pos, centers), CLUSTER_SIZES)


def kernel(features, pos_emb_batch):
    ff = np.asarray(features, dtype=np.float32).reshape(B, C, DF)

    # integrity reference: diag(hi@hi.T) in fp64, cheap on host.  PSUM fp32
    # accumulation keeps the device diagonal within ~0.01 of this; anything
    # larger means a corrupted transfer -> retry the device run once.
    hi64 = ff.astype(np.float16).astype(np.float64)
    diag_ref = np.einsum("bcd,bcd->bc", hi64, hi64)
    for attempt in range(3):
        Ghh, S, _ = _run_device(ff)
        diag_dev = np.einsum("bcc->bc", Ghh.astype(np.float64))
        if np.abs(diag_dev - diag_ref).max() < 0.1:
            break

    ff64 = ff.astype(np.float64)
    n = np.einsum("bcd,bcd->bc", ff64, ff64)
    G = Ghh.astype(np.float64) + S.astype(np.float64) + np.swapaxes(S, 1, 2)
    d2 = n[:, :, None] + n[:, None, :] - 2.0 * G
    return _finish(d2, np.asarray(pos_emb_batch)).astype(np.int32)
